# revision 56
# baseline (speedup 1.0000x reference)
"""Trainium2 Bass kernel for the note/wiki 3-way contraction + gate MLP.

Math (per note n):
    e[n]    = (wikivec * notevec[n]) @ W_emb.T + b_emb          # (C, K)
    attn[n] = sigmoid(e[n] @ W_att.T + b_att)                   # (C, K)
    s[n]    = sum_k attn[n]*e[n]*W_out[0,k] + b_out             # (C,)

Sharding: data-parallel over the 16 notes -> 2 notes per core on 8 cores.
wikivec / W_emb are replicated, pre-transposed to v-major bf16 on the host
and interleaved per 128-row v-tile as [wiki 256 | wemb 256] in one
partition-major dram image so any tile range is a single 2D DMA.

Device phase 1 (per 128-row v-tile): scale the wiki columns by notevec
into a [128, 512] bf16 moving tile (whole tile on ONE engine; DVE takes
11/16 tiles at ~570ns, ACT 5/16 at ~980ns, so the aggregate rate beats
the PE's 426ns/tile), then two matmuls (k-halves) accumulate
e^T[k, (note,c)] into two PSUM banks across all 79 v-tiles.  All chunk
DMAs are issued up-front-ish on the Sync queue (2-tile head chunks then
4-tile chunks); warmup matmuls on a zero tile ramp the PE DVFS clock
during the ~4us DMA prologue.

Phase 2: e+b_emb as bf16 (ACT for k-half 0, DVE PSUM-read for k-half 1,
concurrently), 4 bf16 matmuls for attn logits (jm-major so sigmoid 0
starts early), sigmoid to bf16, gate on DVE at 2x, bf16 W_out matmuls,
+ b_out, DMA out s [1, 512].
"""

import sys

if "/opt/trn_rl_repo" not in sys.path:
    sys.path.insert(0, "/opt/trn_rl_repo")

import numpy as np
import ml_dtypes

import concourse.bass as bass
import concourse.mybir as mybir
import concourse.tile as tile
from concourse import bacc
from concourse.bass_utils import run_bass_kernel_spmd

N_CORES = 8
N, C, V, K = 16, 256, 10000, 256
J = 79  # number of 128-row v-tiles (V=10000 -> 78 full + 1 partial)
J2 = 80  # scales stride per note
NLOC = N // N_CORES  # notes per core
NC2 = NLOC * C  # 512: (note, c) column block
TW = C + K  # 512: combined [wiki | wemb] tile width
BLK = 4  # v-tiles per DMA chunk (small chunks -> smooth arrival pipeline)
# Warmup matmuls keep the PE busy during the DMA prologue so the DVFS
# clock starts ramping before the real stream begins.  Keep the count
# below the PE exec-queue depth (32): filling it wedges the device.
NDUMMY = 10
DUMMY_COLS = 128
# whole-tile producer schedule: each v-tile's two scaling ops run on ONE
# engine; DVE (~572ns/tile) takes 11 of 16, ACT (~980ns/tile) takes 5 of
# 16 at an even spacing of 3, so the aggregate rate beats the PE's
# 426ns/tile and neither engine sits on the critical path.
ACT_TILES = {1, 4, 7, 10, 13}
# All chunk DMAs go on the Sync queue: it has no compute ops, so its
# issue stream runs ahead of the consumers.  (ACT HWDGE queue: blocked
# ~1.3us by ACT_TABLE_LOAD; GpSimd software DGE: ~3.6us per issue.
# Both measured worse.)

F32 = mybir.dt.float32
BF16 = mybir.dt.bfloat16
BF16_NP = ml_dtypes.bfloat16

PACK_W = 167  # 160 scales | 2 wout | 2 bemb | 2 batt | 1 bout

_NC_CACHE = {}


def _build_nc():
    nc = bacc.Bacc(None, target_bir_lowering=False)

    blk_d = nc.declare_dram_parameter("blk", [128, J * TW], BF16, isOutput=False)
    pack_d = nc.declare_dram_parameter("pack", [128, PACK_W], F32, isOutput=False)
    watT = nc.declare_dram_parameter("watT", [128, 2 * K + 2], BF16, isOutput=False)
    s_out = nc.declare_dram_parameter("s_out", [1, NC2], F32, isOutput=True)

    with tile.TileContext(nc) as tc:
        with (
            tc.tile_pool(name="const", bufs=1) as constp,
            tc.tile_pool(name="blk", bufs=6) as blkp,
            tc.tile_pool(name="mov", bufs=8) as movp,
            tc.tile_pool(name="post", bufs=1) as postp,
            tc.tile_pool(name="psum", bufs=1, space="PSUM") as psp,
        ):
            pk = constp.tile([128, PACK_W], F32)
            nc.sync.dma_start(pk[:], pack_d[:])
            sc = pk[:, 0 : NLOC * J2]
            be = pk[:, 162:164]
            ba = pk[:, 164:166]
            bo = pk[0:1, 166:167]

            # PE p-state warmup: matmuls on a zeroed tile keep the PE busy
            # during the DMA prologue so the clock is at full speed when the
            # real stream starts.  GpSimd memset: it starts earliest.
            scr = constp.tile([128, NC2], BF16)
            nc.gpsimd.memset(scr[:], 0.0)
            scr_ps = psp.tile([128, NC2], F32, name="scr_ps", tag="scr_ps")
            for _ in range(NDUMMY):
                nc.tensor.matmul(
                    scr_ps[:, 0:DUMMY_COLS],
                    scr[:, 0:128],
                    scr[:, 0:DUMMY_COLS],
                    start=True,
                    stop=True,
                )

            # Let ACT/DVE observe the pack-DMA semaphore lane up front (the
            # activation engine only supports a single sync-wait per
            # instruction; their first loop op also waits on a block DMA).
            warm0 = constp.tile([128, 1], F32)
            nc.scalar.copy(warm0[:], pk[:, 162:163])
            warmd = constp.tile([128, 1], F32)
            nc.vector.tensor_copy(warmd[:], pk[:, 0:1])

            wat = constp.tile([128, 2 * K + 2], BF16)
            wout = wat[:, 2 * K : 2 * K + 2]

            # e^T accumulators: [k-half 128, (note,c) 512] fp32, one bank each
            e_ps = [
                psp.tile([128, NC2], F32, name=f"e_ps{m}", tag=f"e_ps{m}")
                for m in range(2)
            ]

            # 1-tile then 2-tile head chunks prime the pipeline (first data
            # lands sooner), then 4-tile chunks
            sched = [(0, 1), (1, 1), (2, 2)]
            r = 4
            while r < J:
                sched.append((r, min(BLK, J - r)))
                r += BLK
            for bi, (row0, nval) in enumerate(sched):
                bt = blkp.tile([128, BLK * TW], BF16)
                nc.sync.dma_start(
                    bt[:, 0 : nval * TW],
                    blk_d[:, row0 * TW : (row0 + nval) * TW],
                )
                if bi == 12:
                    # W_att is only needed in phase 2; issue late so it
                    # doesn't delay the block stream.
                    nc.sync.dma_start(wat[:], watT[:])
                for jj in range(nval):
                    g = row0 + jj
                    wts = bt[:, jj * TW : jj * TW + C]
                    emb = bt[:, jj * TW + C : (jj + 1) * TW]
                    mov = movp.tile([128, NC2], BF16)
                    if g % 16 in ACT_TILES:
                        nc.scalar.mul(mov[:, 0:C], wts, mul=sc[:, g : g + 1])
                        nc.scalar.mul(
                            mov[:, C:NC2], wts, mul=sc[:, J2 + g : J2 + g + 1]
                        )
                    else:
                        nc.vector.tensor_scalar_mul(
                            mov[:, 0:C], wts, sc[:, g : g + 1]
                        )
                        nc.vector.tensor_scalar_mul(
                            mov[:, C:NC2], wts, sc[:, J2 + g : J2 + g + 1]
                        )
                    st, sp = (g == 0), (g == J - 1)
                    for m in range(2):
                        nc.tensor.matmul(
                            e_ps[m][:],
                            emb[:, m * 128 : (m + 1) * 128],
                            mov[:],
                            start=st,
                            stop=sp,
                        )


            # ---- phase 2: bias, attn logits, sigmoid, gate, W_out ----
            # eb0 on ACT, eb1 on DVE so they run concurrently
            eb = [
                postp.tile([128, NC2], BF16, name=f"eb{m}", tag=f"eb{m}")
                for m in range(2)
            ]
            nc.scalar.activation(
                eb[0][:],
                e_ps[0][:],
                mybir.ActivationFunctionType.Identity,
                bias=be[:, 0:1],
                scale=1.0,
            )
            nc.vector.tensor_scalar_add(eb[1][:], e_ps[1][:], be[:, 1:2])

            a_ps = [
                psp.tile([128, NC2], F32, name=f"a_ps{jm}", tag=f"a_ps{jm}")
                for jm in range(2)
            ]
            # jm-major: a_ps[0] finishes two matmuls earlier, so sigmoid 0
            # starts sooner
            for jm in range(2):
                for kt in range(2):
                    nc.tensor.matmul(
                        a_ps[jm][:],
                        wat[:, kt * K + jm * 128 : kt * K + (jm + 1) * 128],
                        eb[kt][:],
                        start=(kt == 0),
                        stop=(kt == 1),
                    )

            v = []
            for jm in range(2):
                # bf16 attn: both v-mul inputs 2-byte -> DVE 2x mode
                atn = postp.tile([128, NC2], BF16, tag=f"atn{jm}")
                nc.scalar.activation(
                    atn[:],
                    a_ps[jm][:],
                    mybir.ActivationFunctionType.Sigmoid,
                    bias=ba[:, jm : jm + 1],
                    scale=1.0,
                )
                v_jm = postp.tile([128, NC2], BF16, tag=f"v{jm}")
                nc.vector.tensor_mul(v_jm[:], atn[:], eb[jm][:])
                v.append(v_jm)

            s_ps = psp.tile([1, NC2], F32, tag="s_ps")
            for kt in range(2):
                nc.tensor.matmul(
                    s_ps[:],
                    wout[:, kt : kt + 1],
                    v[kt][:],
                    start=(kt == 0),
                    stop=(kt == 1),
                )
            s_sb = postp.tile([1, NC2], F32, tag="s_sb")
            nc.scalar.activation(
                s_sb[:],
                s_ps[:],
                mybir.ActivationFunctionType.Identity,
                bias=bo[:],
                scale=1.0,
            )
            nc.sync.dma_start(s_out[:], s_sb[:])

    nc.compile()
    return nc


def _get_nc():
    if "nc" not in _NC_CACHE:
        _NC_CACHE["nc"] = _build_nc()
    return _NC_CACHE["nc"]


def prep_inputs(notevec, wikivec, W_emb, b_emb, W_att, b_att, W_out, b_out):
    # blk[g] = [128, 512] bf16: cols 0:256 wikiT rows g*128..g*128+128,
    # cols 256:512 wembT same rows (zero-padded past V)
    both = np.zeros((J * 128, TW), np.float32)
    both[:V, 0:C] = np.asarray(wikivec, np.float32).T
    both[:V, C:TW] = np.asarray(W_emb, np.float32).T
    # partition-major: blk[p, g*TW + c] = tile g, sbuf partition p, col c
    blk = np.ascontiguousarray(
        both.reshape(J, 128, TW).transpose(1, 0, 2).reshape(128, J * TW)
    ).astype(BF16_NP)

    # watT[p, kt*K + j] = W_att[j, kt*128+p]; cols 512:514 hold W_out
    watT = np.zeros((128, 2 * K + 2), np.float32)
    wa = np.asarray(W_att, np.float32)
    for kt in range(2):
        watT[:, kt * K : (kt + 1) * K] = wa[:, kt * 128 : (kt + 1) * 128].T
    watT[:, 2 * K : 2 * K + 2] = np.asarray(W_out, np.float32)[0].reshape(2, 128).T
    watT = np.ascontiguousarray(watT).astype(BF16_NP)

    nv = np.zeros((N, J2 * 128), np.float32)
    nv[:, :V] = np.asarray(notevec, np.float32)

    pack_common = np.zeros((128, PACK_W), np.float32)
    pack_common[:, 162:164] = np.asarray(b_emb, np.float32).reshape(2, 128).T
    pack_common[:, 164:166] = np.asarray(b_att, np.float32).reshape(2, 128).T
    pack_common[:, 166] = np.asarray(b_out, np.float32)[0]

    in_maps = []
    for i in range(N_CORES):
        pack = pack_common.copy()
        # scales[p, l*J2 + g] = notevec[2i+l, g*128+p]
        pack[:, 0 : NLOC * J2] = (
            nv[i * NLOC : (i + 1) * NLOC]
            .reshape(NLOC, J2, 128)
            .transpose(2, 0, 1)
            .reshape(128, NLOC * J2)
        )
        in_maps.append(
            {
                "blk": blk,
                "pack": np.ascontiguousarray(pack),
                "watT": watT,
            }
        )
    return in_maps


def run(in_maps, **kw):
    nc = _get_nc()
    return run_bass_kernel_spmd(nc, in_maps, list(range(N_CORES)), **kw)


def kernel(notevec, wikivec, W_emb, b_emb, W_att, b_att, W_out, b_out):
    in_maps = prep_inputs(
        notevec, wikivec, W_emb, b_emb, W_att, b_att, W_out, b_out
    )
    res = run(in_maps)
    out = np.concatenate(
        [r["s_out"].reshape(NLOC, C) for r in res.results], axis=0
    )
    return out.astype(np.float32)


# revision 59
# speedup vs baseline: 1.0369x; 1.0369x over previous
"""Trainium2 Bass kernel for the note/wiki 3-way contraction + gate MLP.

Math (per note n):
    e[n]    = (wikivec * notevec[n]) @ W_emb.T + b_emb          # (C, K)
    attn[n] = sigmoid(e[n] @ W_att.T + b_att)                   # (C, K)
    s[n]    = sum_k attn[n]*e[n]*W_out[0,k] + b_out             # (C,)

Sharding: data-parallel over the 16 notes -> 2 notes per core on 8 cores.
wikivec / W_emb are replicated, pre-transposed to v-major bf16 on the host
and interleaved per 128-row v-tile as [wiki 256 | wemb 256] in one
partition-major dram image so any tile range is a single 2D DMA.

Device phase 1 (per 128-row v-tile): scale the wiki columns by notevec
into a [128, 512] bf16 moving tile (whole tile on ONE engine; DVE takes
11/16 tiles at ~570ns, ACT 5/16 at ~980ns, so the aggregate rate beats
the PE's 426ns/tile), then two matmuls (k-halves) accumulate
e^T[k, (note,c)] into two PSUM banks across all 79 v-tiles.  All chunk
DMAs are issued up-front-ish on the Sync queue (2-tile head chunks then
4-tile chunks); warmup matmuls on a zero tile ramp the PE DVFS clock
during the ~4us DMA prologue.

Phase 2: e+b_emb as bf16 (ACT for k-half 0, DVE PSUM-read for k-half 1,
concurrently), 4 bf16 matmuls for attn logits (jm-major so sigmoid 0
starts early), sigmoid to bf16, gate on DVE at 2x, bf16 W_out matmuls,
+ b_out, DMA out s [1, 512].
"""

import sys

if "/opt/trn_rl_repo" not in sys.path:
    sys.path.insert(0, "/opt/trn_rl_repo")

import numpy as np
import ml_dtypes

import concourse.bass as bass
import concourse.mybir as mybir
import concourse.tile as tile
from concourse import bacc
from concourse.bass_utils import run_bass_kernel_spmd

N_CORES = 8
N, C, V, K = 16, 256, 10000, 256
J = 79  # number of 128-row v-tiles (V=10000 -> 78 full + 1 partial)
J2 = 80  # scales stride per note
NLOC = N // N_CORES  # notes per core
NC2 = NLOC * C  # 512: (note, c) column block
TW = C + K  # 512: combined [wiki | wemb] tile width
BLK = 4  # v-tiles per DMA chunk (small chunks -> smooth arrival pipeline)
# Warmup matmuls keep the PE busy during the DMA prologue so the DVFS
# clock starts ramping before the real stream begins.  Keep the count
# below the PE exec-queue depth (32): filling it wedges the device.
NDUMMY = 10
DUMMY_COLS = 128
# whole-tile producer schedule: each v-tile's two scaling ops run on ONE
# engine; DVE (~572ns/tile) takes 11 of 16, ACT (~980ns/tile) takes 5 of
# 16 at an even spacing of 3, so the aggregate rate beats the PE's
# 426ns/tile and neither engine sits on the critical path.
ACT_TILES = {1, 4, 7, 10, 13}
# All chunk DMAs go on the Sync queue: it has no compute ops, so its
# issue stream runs ahead of the consumers.  (ACT HWDGE queue: blocked
# ~1.3us by ACT_TABLE_LOAD; GpSimd software DGE: ~3.6us per issue.
# Both measured worse.)

F32 = mybir.dt.float32
BF16 = mybir.dt.bfloat16
BF16_NP = ml_dtypes.bfloat16

PACK_W = 167  # 160 scales | 2 wout | 2 bemb | 2 batt | 1 bout

_NC_CACHE = {}


def _build_nc():
    nc = bacc.Bacc(None, target_bir_lowering=False)

    blk_d = nc.declare_dram_parameter("blk", [128, J * TW], BF16, isOutput=False)
    pack_d = nc.declare_dram_parameter("pack", [128, PACK_W], F32, isOutput=False)
    watT = nc.declare_dram_parameter("watT", [128, 2 * K + 2], BF16, isOutput=False)
    s_out = nc.declare_dram_parameter("s_out", [1, NC2], F32, isOutput=True)

    with tile.TileContext(nc) as tc:
        with (
            tc.tile_pool(name="const", bufs=1) as constp,
            tc.tile_pool(name="blk", bufs=6) as blkp,
            tc.tile_pool(name="mov", bufs=6) as movp,
            tc.tile_pool(name="post", bufs=1) as postp,
            tc.tile_pool(name="psum", bufs=1, space="PSUM") as psp,
        ):
            pk = constp.tile([128, PACK_W], F32)
            sc = pk[:, 0 : NLOC * J2]
            be = pk[:, 162:164]
            ba = pk[:, 164:166]
            bo = pk[0:1, 166:167]

            # PE p-state warmup: matmuls on a zeroed tile keep the PE busy
            # during the DMA prologue so the clock is at full speed when the
            # real stream starts.  GpSimd memset: it starts earliest.
            scr = constp.tile([128, NC2], BF16)
            nc.gpsimd.memset(scr[:], 0.0)
            scr_ps = psp.tile([128, NC2], F32, name="scr_ps", tag="scr_ps")
            for _ in range(NDUMMY):
                nc.tensor.matmul(
                    scr_ps[:, 0:DUMMY_COLS],
                    scr[:, 0:128],
                    scr[:, 0:DUMMY_COLS],
                    start=True,
                    stop=True,
                )

            # Let ACT/DVE observe the pack-DMA semaphore lane up front (the
            # activation engine only supports a single sync-wait per
            # instruction; their first loop op also waits on a block DMA).
            warm0 = constp.tile([128, 1], F32)
            nc.scalar.copy(warm0[:], pk[:, 162:163])
            warmd = constp.tile([128, 1], F32)
            nc.vector.tensor_copy(warmd[:], pk[:, 0:1])

            wat = constp.tile([128, 2 * K + 2], BF16)
            wout = wat[:, 2 * K : 2 * K + 2]

            # e^T accumulators: [k-half 128, (note,c) 512] fp32, one bank each
            e_ps = [
                psp.tile([128, NC2], F32, name=f"e_ps{m}", tag=f"e_ps{m}")
                for m in range(2)
            ]

            # two 2-tile head chunks prime the pipeline, then 4-tile chunks
            sched = [(0, 2), (2, 2)]
            r = 4
            while r < J:
                sched.append((r, min(BLK, J - r)))
                r += BLK
            for bi, (row0, nval) in enumerate(sched):
                bt = blkp.tile([128, BLK * TW], BF16)
                nc.sync.dma_start(
                    bt[:, 0 : nval * TW],
                    blk_d[:, row0 * TW : (row0 + nval) * TW],
                )
                if bi == 0:
                    # pack issued after chunk 0: the first mul gates on
                    # max(pack, c0) and c0's transfer is the longer pole
                    nc.sync.dma_start(pk[:], pack_d[:])
                    # W_att rides the idle ACT HWDGE queue (not needed
                    # until phase 2), freeing a Sync issue slot
                    nc.scalar.dma_start(wat[:], watT[:])
                for jj in range(nval):
                    g = row0 + jj
                    wts = bt[:, jj * TW : jj * TW + C]
                    emb = bt[:, jj * TW + C : (jj + 1) * TW]
                    mov = movp.tile([128, NC2], BF16)
                    if g % 16 in ACT_TILES:
                        nc.scalar.mul(mov[:, 0:C], wts, mul=sc[:, g : g + 1])
                        nc.scalar.mul(
                            mov[:, C:NC2], wts, mul=sc[:, J2 + g : J2 + g + 1]
                        )
                    else:
                        nc.vector.tensor_scalar_mul(
                            mov[:, 0:C], wts, sc[:, g : g + 1]
                        )
                        nc.vector.tensor_scalar_mul(
                            mov[:, C:NC2], wts, sc[:, J2 + g : J2 + g + 1]
                        )
                    st, sp = (g == 0), (g == J - 1)
                    for m in range(2):
                        nc.tensor.matmul(
                            e_ps[m][:],
                            emb[:, m * 128 : (m + 1) * 128],
                            mov[:],
                            start=st,
                            stop=sp,
                        )


            # ---- phase 2: bias, attn logits, sigmoid, gate, W_out ----
            # eb0 on ACT, eb1 on DVE so they run concurrently
            eb = [
                postp.tile([128, NC2], BF16, name=f"eb{m}", tag=f"eb{m}")
                for m in range(2)
            ]
            nc.scalar.activation(
                eb[0][:],
                e_ps[0][:],
                mybir.ActivationFunctionType.Identity,
                bias=be[:, 0:1],
                scale=1.0,
            )
            nc.vector.tensor_scalar_add(eb[1][:], e_ps[1][:], be[:, 1:2])

            a_ps = [
                psp.tile([128, NC2], F32, name=f"a_ps{jm}", tag=f"a_ps{jm}")
                for jm in range(2)
            ]
            # jm-major: a_ps[0] finishes two matmuls earlier, so sigmoid 0
            # starts sooner
            for jm in range(2):
                for kt in range(2):
                    nc.tensor.matmul(
                        a_ps[jm][:],
                        wat[:, kt * K + jm * 128 : kt * K + (jm + 1) * 128],
                        eb[kt][:],
                        start=(kt == 0),
                        stop=(kt == 1),
                    )

            v = []
            for jm in range(2):
                # bf16 attn: both v-mul inputs 2-byte -> DVE 2x mode
                atn = postp.tile([128, NC2], BF16, tag=f"atn{jm}")
                nc.scalar.activation(
                    atn[:],
                    a_ps[jm][:],
                    mybir.ActivationFunctionType.Sigmoid,
                    bias=ba[:, jm : jm + 1],
                    scale=1.0,
                )
                v_jm = postp.tile([128, NC2], BF16, tag=f"v{jm}")
                nc.vector.tensor_mul(v_jm[:], atn[:], eb[jm][:])
                v.append(v_jm)

            s_ps = psp.tile([1, NC2], F32, tag="s_ps")
            for kt in range(2):
                nc.tensor.matmul(
                    s_ps[:],
                    wout[:, kt : kt + 1],
                    v[kt][:],
                    start=(kt == 0),
                    stop=(kt == 1),
                )
            s_sb = postp.tile([1, NC2], F32, tag="s_sb")
            nc.scalar.activation(
                s_sb[:],
                s_ps[:],
                mybir.ActivationFunctionType.Identity,
                bias=bo[:],
                scale=1.0,
            )
            nc.sync.dma_start(s_out[:], s_sb[:])

    nc.compile()
    return nc


def _get_nc():
    if "nc" not in _NC_CACHE:
        _NC_CACHE["nc"] = _build_nc()
    return _NC_CACHE["nc"]


def prep_inputs(notevec, wikivec, W_emb, b_emb, W_att, b_att, W_out, b_out):
    # blk[g] = [128, 512] bf16: cols 0:256 wikiT rows g*128..g*128+128,
    # cols 256:512 wembT same rows (zero-padded past V)
    both = np.zeros((J * 128, TW), np.float32)
    both[:V, 0:C] = np.asarray(wikivec, np.float32).T
    both[:V, C:TW] = np.asarray(W_emb, np.float32).T
    # partition-major: blk[p, g*TW + c] = tile g, sbuf partition p, col c
    blk = np.ascontiguousarray(
        both.reshape(J, 128, TW).transpose(1, 0, 2).reshape(128, J * TW)
    ).astype(BF16_NP)

    # watT[p, kt*K + j] = W_att[j, kt*128+p]; cols 512:514 hold W_out
    watT = np.zeros((128, 2 * K + 2), np.float32)
    wa = np.asarray(W_att, np.float32)
    for kt in range(2):
        watT[:, kt * K : (kt + 1) * K] = wa[:, kt * 128 : (kt + 1) * 128].T
    watT[:, 2 * K : 2 * K + 2] = np.asarray(W_out, np.float32)[0].reshape(2, 128).T
    watT = np.ascontiguousarray(watT).astype(BF16_NP)

    nv = np.zeros((N, J2 * 128), np.float32)
    nv[:, :V] = np.asarray(notevec, np.float32)

    pack_common = np.zeros((128, PACK_W), np.float32)
    pack_common[:, 162:164] = np.asarray(b_emb, np.float32).reshape(2, 128).T
    pack_common[:, 164:166] = np.asarray(b_att, np.float32).reshape(2, 128).T
    pack_common[:, 166] = np.asarray(b_out, np.float32)[0]

    in_maps = []
    for i in range(N_CORES):
        pack = pack_common.copy()
        # scales[p, l*J2 + g] = notevec[2i+l, g*128+p]
        pack[:, 0 : NLOC * J2] = (
            nv[i * NLOC : (i + 1) * NLOC]
            .reshape(NLOC, J2, 128)
            .transpose(2, 0, 1)
            .reshape(128, NLOC * J2)
        )
        in_maps.append(
            {
                "blk": blk,
                "pack": np.ascontiguousarray(pack),
                "watT": watT,
            }
        )
    return in_maps


def run(in_maps, **kw):
    nc = _get_nc()
    return run_bass_kernel_spmd(nc, in_maps, list(range(N_CORES)), **kw)


def kernel(notevec, wikivec, W_emb, b_emb, W_att, b_att, W_out, b_out):
    in_maps = prep_inputs(
        notevec, wikivec, W_emb, b_emb, W_att, b_att, W_out, b_out
    )
    res = run(in_maps)
    out = np.concatenate(
        [r["s_out"].reshape(NLOC, C) for r in res.results], axis=0
    )
    return out.astype(np.float32)
